# Initial kernel scaffold
#
"""GCNII-with-JK distributed Trainium2 kernel (8 NeuronCores).

Strategy (hardcoded for N=100000, E=1600000, H=128, L=8):
  - Nodes dst-sharded across 8 cores (12500/core, padded to 12544 = 98 windows x 128).
  - Per-core node->window assignment balanced by degree (LPT) so every window's
    edge list fits n_slots chunks of 128 edges (host-precomputed index tables).
  - Edge gather: gpsimd indirect DMA from a DRAM z-table (replicated via AllGather
    each layer); 1 instruction per window (n_slots*128 descriptors).
  - Segment-sum scatter: one-hot matmuls accumulated in PSUM.
      layer0 (GCNConv) scheme A: out = onehot^T @ G -> [dst, feat] node-major.
      layers 1..8 scheme B:      out = G^T @ onehot -> [feat, dst] feature-major,
      feeding z = (0.9*agg + 0.1*x0) @ W'_i as two accumulated matmuls with
      host-folded weights W'_i = (1-beta_i) I + beta_i conv_w[i].
  - BN(eval)+relu folded as replicated row constants; JK 'max' every 4 layers.
"""
import sys
sys.path.insert(0, "/opt/trn_rl_repo")
import hashlib
import heapq
import numpy as np

N, E, H, L = 100000, 1600000, 128, 8
ALPHA, THETA, BN_EPS = 0.1, 0.5, 1e-5
C = 8
NS = N // C          # 12500
P = 128
NW = 98              # windows per core
NS_PAD = NW * P      # 12544
NT = C * NS_PAD      # 100352 table rows


# ----------------------------------------------------------------- host prep
def _host_prep(edge_index):
    src = np.asarray(edge_index[0]).astype(np.int64)
    dst = np.asarray(edge_index[1]).astype(np.int64)
    deg = np.bincount(dst, minlength=N).astype(np.float32) + 1.0
    dinv = (1.0 / np.sqrt(deg)).astype(np.float32)

    # balanced node->(window,slot) assignment per core (LPT on degree)
    perm_pos = np.empty(N, np.int64)
    for c in range(C):
        nodes = np.arange(c * NS, (c + 1) * NS)
        d = deg[nodes] - 1.0
        order = np.argsort(-d, kind="stable")
        wcnt = np.zeros(NW, np.int64)
        heap = [(0.0, w) for w in range(NW)]
        heapq.heapify(heap)
        pos = np.empty(NS, np.int64)
        for n_i in order:
            while True:
                s, w = heapq.heappop(heap)
                if wcnt[w] < P:
                    break
            pos[n_i] = w * P + wcnt[w]
            wcnt[w] += 1
            heapq.heappush(heap, (s + d[n_i], w))
        perm_pos[nodes] = c * NS_PAD + pos

    src_pos = perm_pos[src]
    dst_pos = perm_pos[dst]
    dst_core = dst_pos // NS_PAD
    e_w = (dst_pos % NS_PAD) // P
    e_slot = dst_pos % P

    # group edges by (core, window); assign chunk/lane positions
    key = dst_core * NW + e_w
    order = np.argsort(key, kind="stable")
    key_s = key[order]
    grp_start = np.searchsorted(key_s, np.arange(C * NW))
    grp_end = np.searchsorted(key_s, np.arange(C * NW) + 1)
    counts = grp_end - grp_start
    n_slots = int((counts.max() + P - 1) // P)
    NSL = NW * n_slots

    eidx = np.zeros((C, P, NSL), np.int32)
    dstloc = np.full((C, P, NSL), -1.0, np.float32)
    k_in_grp = np.arange(E) - grp_start[key_s]          # rank within group
    ch = k_in_grp // P
    lane = k_in_grp % P
    g_c = key_s // NW
    g_w = key_s % NW
    col = g_w * n_slots + ch
    eidx[g_c, lane, col] = src_pos[order]
    dstloc[g_c, lane, col] = e_slot[order].astype(np.float32)

    dinv_pad = np.ones((C, P, NW), np.float32)
    lp = perm_pos % NS_PAD
    dinv_pad[perm_pos // NS_PAD, lp % P, lp // P] = dinv

    return dict(perm_pos=perm_pos, eidx=eidx, dstloc=dstloc, dinv=dinv_pad,
                n_slots=n_slots)


def _host_consts(inputs):
    w0 = np.asarray(inputs["w0"], np.float32)
    b0 = np.asarray(inputs["b0"], np.float32)
    conv_w = np.asarray(inputs["conv_w"], np.float32)
    bn_gamma = np.asarray(inputs["bn_gamma"], np.float32)
    bn_beta = np.asarray(inputs["bn_beta"], np.float32)
    bn_scale = bn_gamma / np.float32(np.sqrt(1.0 + BN_EPS))

    wp09 = np.zeros((P, L * P), np.float32)
    wp01 = np.zeros((P, L * P), np.float32)
    bn_s = np.zeros((P, L * P), np.float32)
    bn_b = np.zeros((P, L * P), np.float32)
    eye = np.eye(H, dtype=np.float32)
    for i in range(L):
        beta = np.float32(np.log(THETA / (i + 1) + 1.0))
        Wp = (1.0 - beta) * eye + beta * conv_w[i]
        wp09[:, i * P:(i + 1) * P] = np.float32(1.0 - ALPHA) * Wp
        wp01[:, i * P:(i + 1) * P] = np.float32(ALPHA) * Wp
        bn_s[:, i * P:(i + 1) * P] = np.broadcast_to(bn_scale[i], (P, H))
        bn_b[:, i * P:(i + 1) * P] = np.broadcast_to(bn_beta[i], (P, H))
    b0r = np.broadcast_to(b0, (P, H)).astype(np.float32).copy()
    iota = np.broadcast_to(np.arange(P, dtype=np.float32), (P, P)).copy()
    return dict(w0=w0, wp09=wp09, wp01=wp01, bn_s=bn_s, bn_b=bn_b, b0r=b0r,
                iota=iota)


# ------------------------------------------------------------ device program
def _build_program(n_slots):
    from concourse import bass, bacc, mybir, tile
    from concourse.masks import make_identity

    f32 = mybir.dt.float32
    i32 = mybir.dt.int32
    Alu = mybir.AluOpType
    Act = mybir.ActivationFunctionType
    NSL = NW * n_slots

    nc = bacc.Bacc("TRN2", target_bir_lowering=False, debug=False, num_devices=C)

    # external I/O
    xs_io = nc.dram_tensor("xs", [NS_PAD, H], f32, kind="ExternalInput")
    eidx_io = nc.dram_tensor("eidx", [P, NSL], i32, kind="ExternalInput")
    dstloc_io = nc.dram_tensor("dstloc", [P, NSL], f32, kind="ExternalInput")
    dinv_io = nc.dram_tensor("dinv", [P, NW], f32, kind="ExternalInput")
    w0_io = nc.dram_tensor("w0", [P, H], f32, kind="ExternalInput")
    wp09_io = nc.dram_tensor("wp09", [P, L * P], f32, kind="ExternalInput")
    wp01_io = nc.dram_tensor("wp01", [P, L * P], f32, kind="ExternalInput")
    bn_s_io = nc.dram_tensor("bn_s", [P, L * P], f32, kind="ExternalInput")
    bn_b_io = nc.dram_tensor("bn_b", [P, L * P], f32, kind="ExternalInput")
    b0r_io = nc.dram_tensor("b0r", [P, H], f32, kind="ExternalInput")
    iota_io = nc.dram_tensor("iota", [P, P], f32, kind="ExternalInput")
    out_io = nc.dram_tensor("out", [NS_PAD, H], f32, kind="ExternalOutput")

    with tile.TileContext(nc) as tc:
        with (
            tc.tile_pool(name="const", bufs=1) as cpool,
            tc.tile_pool(name="big", bufs=1) as bigpool,
            tc.tile_pool(name="gbuf", bufs=2) as gpool,
            tc.tile_pool(name="win", bufs=3) as wpool,
            tc.tile_pool(name="psA", bufs=2, space="PSUM") as psA,
            tc.tile_pool(name="psB", bufs=2, space="PSUM") as psB,
            tc.tile_pool(name="psC", bufs=2, space="PSUM") as psC,
            tc.tile_pool(name="dram", bufs=1, space="DRAM") as dram,
        ):
            # ---- constants to SBUF
            eidx_t = cpool.tile([P, NSL], i32, name="eidx_t")
            dstloc_t = cpool.tile([P, NSL], f32, name="dstloc_t")
            dinv_t = cpool.tile([P, NW], f32, name="dinv_t")
            w0_t = cpool.tile([P, H], f32, name="w0_t")
            wp09_t = cpool.tile([P, L * P], f32, name="wp09_t")
            wp01_t = cpool.tile([P, L * P], f32, name="wp01_t")
            bn_s_t = cpool.tile([P, L * P], f32, name="bn_s_t")
            bn_b_t = cpool.tile([P, L * P], f32, name="bn_b_t")
            b0r_t = cpool.tile([P, H], f32, name="b0r_t")
            iota_t = cpool.tile([P, P], f32, name="iota_t")
            ident_t = cpool.tile([P, P], f32, name="ident_t")
            for t, io in [(eidx_t, eidx_io), (dstloc_t, dstloc_io),
                          (dinv_t, dinv_io), (w0_t, w0_io), (wp09_t, wp09_io),
                          (wp01_t, wp01_io), (bn_s_t, bn_s_io), (bn_b_t, bn_b_io),
                          (b0r_t, b0r_io), (iota_t, iota_io)]:
                nc.sync.dma_start(t[:], io[:])
            make_identity(nc, ident_t[:])

            x0T_a = bigpool.tile([P, NS_PAD], f32, name="x0T_a")
            x0T_b = bigpool.tile([P, NS_PAD], f32, name="x0T_b")

            tables = [dram.tile([NT, H], f32, addr_space="Shared", name=f"table{i}")
                      for i in range(L + 1)]
            agbufs = [dram.tile([NS_PAD, H], f32, name=f"agbuf{i}")
                      for i in range(L + 1)]
            zsbufs = {i: dram.tile([NS_PAD, H], f32, name=f"zsbuf{i}")
                      for i in (0, 1, 2, 4, 5, 6)}
            hd2buf = dram.tile([NS_PAD, H], f32, name="hd2buf")

            RG = [list(range(C))]

            def allgather(i):
                nc.gpsimd.collective_compute(
                    "AllGather", Alu.bypass, replica_groups=RG,
                    ins=[agbufs[i].opt()], outs=[tables[i].opt()])

            def gather_window(table, w):
                g = gpool.tile([P, n_slots * P], f32, name="g")
                nc.gpsimd.indirect_dma_start(
                    out=g[:], out_offset=None, in_=table[:],
                    in_offset=bass.IndirectOffsetOnAxis(
                        ap=eidx_t[:, w * n_slots:(w + 1) * n_slots], axis=0))
                return g

            def onehot_window(w):
                oh = gpool.tile([P, n_slots * P], f32, name="oh")
                src = dstloc_t[:, w * n_slots:(w + 1) * n_slots]
                in0 = src.to_broadcast([P, n_slots, P])
                io_ap = iota_t[:]
                in1 = bass.AP(io_ap.tensor, io_ap.offset,
                              [list(io_ap.ap[0]), [0, n_slots], [1, P]])
                nc.vector.tensor_tensor(out=oh[:], in0=in0, in1=in1, op=Alu.is_equal)
                return oh

            # ================= Phase A: h' = (x @ w0) * dinv =================
            for w in range(NW):
                ws = slice(w * P, (w + 1) * P)
                xw = wpool.tile([P, H], f32, name="xw")
                nc.sync.dma_start(xw[:], xs_io[ws])
                xT_ps = psA.tile([P, P], f32, name="xT_ps")
                nc.tensor.transpose(out=xT_ps[:], in_=xw[:], identity=ident_t[:])
                xT = wpool.tile([P, P], f32, name="xT")
                nc.vector.tensor_copy(out=xT[:], in_=xT_ps[:])
                h_ps = psB.tile([P, H], f32, name="h_ps")
                nc.tensor.matmul(out=h_ps[:], lhsT=xT[:], rhs=w0_t[:],
                                 start=True, stop=True)
                dcol = dinv_t[:, w:w + 1]
                hp = wpool.tile([P, H], f32, name="hp")
                nc.vector.tensor_scalar_mul(hp[:], h_ps[:], dcol)
                hd2b = wpool.tile([P, H], f32, name="hd2b")
                nc.vector.scalar_tensor_tensor(
                    out=hd2b[:], in0=hp[:], scalar=dcol, in1=b0r_t[:],
                    op0=Alu.mult, op1=Alu.add)
                nc.sync.dma_start(agbufs[0][ws], hp[:])
                nc.scalar.dma_start(hd2buf[ws], hd2b[:])
            allgather(0)

            # ============ Phase B: z0 = dinv*segsum(h'[src]) + h*dinv^2 + b0
            for w in range(NW):
                ws = slice(w * P, (w + 1) * P)
                g = gather_window(tables[0], w)
                oh = onehot_window(w)
                s_ps = psA.tile([P, H], f32, name="s_ps")
                for ci in range(n_slots):
                    cs = slice(ci * P, (ci + 1) * P)
                    nc.tensor.matmul(out=s_ps[:], lhsT=oh[:, cs], rhs=g[:, cs],
                                     start=(ci == 0), stop=(ci == n_slots - 1))
                hd2w = wpool.tile([P, H], f32, name="hd2w")
                nc.scalar.dma_start(hd2w[:], hd2buf[ws])
                z0 = wpool.tile([P, H], f32, name="z0")
                nc.vector.scalar_tensor_tensor(
                    out=z0[:], in0=s_ps[:], scalar=dinv_t[:, w:w + 1], in1=hd2w[:],
                    op0=Alu.mult, op1=Alu.add)
                nc.sync.dma_start(agbufs[1][ws], z0[:])
                zT_ps = psB.tile([P, P], f32, name="zT_ps")
                nc.tensor.transpose(out=zT_ps[:], in_=z0[:], identity=ident_t[:])
                nc.vector.tensor_copy(out=x0T_a[:, ws], in_=zT_ps[:])
            allgather(1)

            # =========================== Phase C: 8 GCN2 layers
            for i in range(L):
                x0T = x0T_a if i < 4 else x0T_b
                lsl = slice(i * P, (i + 1) * P)
                for w in range(NW):
                    ws = slice(w * P, (w + 1) * P)
                    g = gather_window(tables[i + 1], w)
                    oh = onehot_window(w)
                    st_ps = psA.tile([P, P], f32, name="st_ps")
                    for ci in range(n_slots):
                        cs = slice(ci * P, (ci + 1) * P)
                        nc.tensor.matmul(out=st_ps[:], lhsT=g[:, cs], rhs=oh[:, cs],
                                         start=(ci == 0), stop=(ci == n_slots - 1))
                    st = wpool.tile([P, P], f32, name="st")
                    nc.vector.tensor_copy(out=st[:], in_=st_ps[:])
                    z_ps = psB.tile([P, H], f32, name="z_ps")
                    nc.tensor.matmul(out=z_ps[:], lhsT=st[:], rhs=wp09_t[:, lsl],
                                     start=True, stop=False)
                    nc.tensor.matmul(out=z_ps[:], lhsT=x0T[:, ws], rhs=wp01_t[:, lsl],
                                     start=False, stop=True)
                    if i in (3, 7):
                        m = wpool.tile([P, H], f32, name="m")
                        nc.vector.tensor_copy(out=m[:], in_=z_ps[:])
                        for j in range(4 * (i // 4), 4 * (i // 4) + 3):
                            zl = wpool.tile([P, H], f32, name="zl")
                            nc.scalar.dma_start(zl[:], zsbufs[j][ws])
                            nc.vector.tensor_max(m[:], m[:], zl[:])
                        if i == 3:
                            nc.sync.dma_start(agbufs[i + 2][ws], m[:])
                            mT_ps = psC.tile([P, P], f32, name="mT_ps")
                            nc.tensor.transpose(out=mT_ps[:], in_=m[:],
                                                identity=ident_t[:])
                            nc.vector.tensor_copy(out=x0T_b[:, ws], in_=mT_ps[:])
                        else:
                            nc.sync.dma_start(out_io[ws], m[:])
                    else:
                        zsb = wpool.tile([P, H], f32, name="zsb")
                        nc.vector.tensor_copy(out=zsb[:], in_=z_ps[:])
                        nc.scalar.dma_start(zsbufs[i][ws], zsb[:])
                        t1 = wpool.tile([P, H], f32, name="t1")
                        nc.vector.tensor_tensor(out=t1[:], in0=z_ps[:],
                                                in1=bn_s_t[:, lsl], op=Alu.mult)
                        t2 = wpool.tile([P, H], f32, name="t2")
                        nc.vector.tensor_tensor(out=t2[:], in0=t1[:],
                                                in1=bn_b_t[:, lsl], op=Alu.add)
                        za = wpool.tile([P, H], f32, name="za")
                        nc.scalar.activation(out=za[:], in_=t2[:], func=Act.Relu)
                        nc.sync.dma_start(agbufs[i + 2][ws], za[:])
                if i < 7:
                    allgather(i + 2)
    nc.finalize()
    return nc


_PROGRAM_CACHE = {}
_PREP_CACHE = {}


def kernel(**inputs) -> np.ndarray:
    from concourse.bass_utils import run_bass_kernel_spmd

    edge_index = np.asarray(inputs["edge_index"])
    ekey = hashlib.md5(edge_index.tobytes()).hexdigest()
    if ekey not in _PREP_CACHE:
        _PREP_CACHE[ekey] = _host_prep(edge_index)
    prep = _PREP_CACHE[ekey]
    n_slots = prep["n_slots"]
    if n_slots not in _PROGRAM_CACHE:
        _PROGRAM_CACHE[n_slots] = _build_program(n_slots)
    nc = _PROGRAM_CACHE[n_slots]

    consts = _host_consts(inputs)
    x = np.asarray(inputs["x"], np.float32)
    xp = np.zeros((C * NS_PAD, H), np.float32)
    xp[prep["perm_pos"]] = x

    in_maps = []
    for c in range(C):
        in_maps.append({
            "xs": xp[c * NS_PAD:(c + 1) * NS_PAD],
            "eidx": prep["eidx"][c],
            "dstloc": prep["dstloc"][c],
            "dinv": prep["dinv"][c],
            "w0": consts["w0"], "wp09": consts["wp09"], "wp01": consts["wp01"],
            "bn_s": consts["bn_s"], "bn_b": consts["bn_b"], "b0r": consts["b0r"],
            "iota": consts["iota"],
        })
    res = run_bass_kernel_spmd(nc, in_maps, list(range(C)))
    out_cat = np.concatenate([res.results[c]["out"] for c in range(C)], axis=0)
    return out_cat[prep["perm_pos"]]


# revision 2
# speedup vs baseline: 1.0459x; 1.0459x over previous
"""GCNII-with-JK distributed Trainium2 kernel (8 NeuronCores).

Strategy (hardcoded for N=100000, E=1600000, H=128, L=8):
  - Nodes dst-sharded across 8 cores (12500/core, padded to 12544 = 98 windows x 128).
  - Per-core node->window assignment balanced by degree (LPT) so every window's
    edge list fits n_slots chunks of 128 edges (host-precomputed index tables).
  - Edge gather: gpsimd indirect DMA from a DRAM z-table (replicated via AllGather
    each layer); 1 instruction per window (n_slots*128 descriptors).
  - Segment-sum scatter: one-hot matmuls accumulated in PSUM.
      layer0 (GCNConv) scheme A: out = onehot^T @ G -> [dst, feat] node-major.
      layers 1..8 scheme B:      out = G^T @ onehot -> [feat, dst] feature-major,
      feeding z = (0.9*agg + 0.1*x0) @ W'_i as two accumulated matmuls with
      host-folded weights W'_i = (1-beta_i) I + beta_i conv_w[i].
  - BN(eval)+relu folded as replicated row constants; JK 'max' every 4 layers.
"""
import sys
sys.path.insert(0, "/opt/trn_rl_repo")
import hashlib
import heapq
import numpy as np

N, E, H, L = 100000, 1600000, 128, 8
ALPHA, THETA, BN_EPS = 0.1, 0.5, 1e-5
C = 8
NS = N // C          # 12500
P = 128
NW = 98              # windows per core
NS_PAD = NW * P      # 12544
NT = C * NS_PAD      # 100352 table rows


# ----------------------------------------------------------------- host prep
def _host_prep(edge_index):
    src = np.asarray(edge_index[0]).astype(np.int64)
    dst = np.asarray(edge_index[1]).astype(np.int64)
    deg = np.bincount(dst, minlength=N).astype(np.float32) + 1.0
    dinv = (1.0 / np.sqrt(deg)).astype(np.float32)

    # balanced node->(window,slot) assignment per core (LPT on degree)
    perm_pos = np.empty(N, np.int64)
    for c in range(C):
        nodes = np.arange(c * NS, (c + 1) * NS)
        d = deg[nodes] - 1.0
        order = np.argsort(-d, kind="stable")
        wcnt = np.zeros(NW, np.int64)
        heap = [(0.0, w) for w in range(NW)]
        heapq.heapify(heap)
        pos = np.empty(NS, np.int64)
        for n_i in order:
            while True:
                s, w = heapq.heappop(heap)
                if wcnt[w] < P:
                    break
            pos[n_i] = w * P + wcnt[w]
            wcnt[w] += 1
            heapq.heappush(heap, (s + d[n_i], w))
        perm_pos[nodes] = c * NS_PAD + pos

    src_pos = perm_pos[src]
    dst_pos = perm_pos[dst]
    dst_core = dst_pos // NS_PAD
    e_w = (dst_pos % NS_PAD) // P
    e_slot = dst_pos % P

    # group edges by (core, window); assign chunk/lane positions
    key = dst_core * NW + e_w
    order = np.argsort(key, kind="stable")
    key_s = key[order]
    grp_start = np.searchsorted(key_s, np.arange(C * NW))
    grp_end = np.searchsorted(key_s, np.arange(C * NW) + 1)
    counts = grp_end - grp_start
    n_slots = int((counts.max() + P - 1) // P)
    NSL = NW * n_slots

    eidx = np.zeros((C, P, NSL), np.int32)
    dstloc = np.full((C, P, NSL), -1.0, np.float32)
    k_in_grp = np.arange(E) - grp_start[key_s]          # rank within group
    ch = k_in_grp // P
    lane = k_in_grp % P
    g_c = key_s // NW
    g_w = key_s % NW
    col = g_w * n_slots + ch
    eidx[g_c, lane, col] = src_pos[order]
    dstloc[g_c, lane, col] = e_slot[order].astype(np.float32)

    dinv_pad = np.ones((C, P, NW), np.float32)
    lp = perm_pos % NS_PAD
    dinv_pad[perm_pos // NS_PAD, lp % P, lp // P] = dinv

    return dict(perm_pos=perm_pos, eidx=eidx, dstloc=dstloc, dinv=dinv_pad,
                n_slots=n_slots)


def _host_consts(inputs):
    w0 = np.asarray(inputs["w0"], np.float32)
    b0 = np.asarray(inputs["b0"], np.float32)
    conv_w = np.asarray(inputs["conv_w"], np.float32)
    bn_gamma = np.asarray(inputs["bn_gamma"], np.float32)
    bn_beta = np.asarray(inputs["bn_beta"], np.float32)
    bn_scale = bn_gamma / np.float32(np.sqrt(1.0 + BN_EPS))

    wp09 = np.zeros((P, L * P), np.float32)
    wp01 = np.zeros((P, L * P), np.float32)
    bn_s = np.zeros((P, L * P), np.float32)
    bn_b = np.zeros((P, L * P), np.float32)
    eye = np.eye(H, dtype=np.float32)
    for i in range(L):
        beta = np.float32(np.log(THETA / (i + 1) + 1.0))
        Wp = (1.0 - beta) * eye + beta * conv_w[i]
        wp09[:, i * P:(i + 1) * P] = np.float32(1.0 - ALPHA) * Wp
        wp01[:, i * P:(i + 1) * P] = np.float32(ALPHA) * Wp
        bn_s[:, i * P:(i + 1) * P] = np.broadcast_to(bn_scale[i], (P, H))
        bn_b[:, i * P:(i + 1) * P] = np.broadcast_to(bn_beta[i], (P, H))
    b0r = np.broadcast_to(b0, (P, H)).astype(np.float32).copy()
    iota = np.broadcast_to(np.arange(P, dtype=np.float32), (P, P)).copy()
    return dict(w0=w0, wp09=wp09, wp01=wp01, bn_s=bn_s, bn_b=bn_b, b0r=b0r,
                iota=iota)


# ------------------------------------------------------------ device program
def _build_program(n_slots):
    from concourse import bass, bacc, mybir, tile
    from concourse.masks import make_identity

    f32 = mybir.dt.float32
    i32 = mybir.dt.int32
    Alu = mybir.AluOpType
    Act = mybir.ActivationFunctionType
    NSL = NW * n_slots

    nc = bacc.Bacc("TRN2", target_bir_lowering=False, debug=False, num_devices=C)

    # external I/O
    xs_io = nc.dram_tensor("xs", [NS_PAD, H], f32, kind="ExternalInput")
    eidx_io = nc.dram_tensor("eidx", [P, NSL], i32, kind="ExternalInput")
    dstloc_io = nc.dram_tensor("dstloc", [P, NSL], f32, kind="ExternalInput")
    dinv_io = nc.dram_tensor("dinv", [P, NW], f32, kind="ExternalInput")
    w0_io = nc.dram_tensor("w0", [P, H], f32, kind="ExternalInput")
    wp09_io = nc.dram_tensor("wp09", [P, L * P], f32, kind="ExternalInput")
    wp01_io = nc.dram_tensor("wp01", [P, L * P], f32, kind="ExternalInput")
    bn_s_io = nc.dram_tensor("bn_s", [P, L * P], f32, kind="ExternalInput")
    bn_b_io = nc.dram_tensor("bn_b", [P, L * P], f32, kind="ExternalInput")
    b0r_io = nc.dram_tensor("b0r", [P, H], f32, kind="ExternalInput")
    iota_io = nc.dram_tensor("iota", [P, P], f32, kind="ExternalInput")
    out_io = nc.dram_tensor("out", [NS_PAD, H], f32, kind="ExternalOutput")

    with tile.TileContext(nc) as tc:
        with (
            tc.tile_pool(name="const", bufs=1) as cpool,
            tc.tile_pool(name="big", bufs=1) as bigpool,
            tc.tile_pool(name="gbuf", bufs=2) as gpool,
            tc.tile_pool(name="win", bufs=3) as wpool,
            tc.tile_pool(name="ps", bufs=2, space="PSUM") as ps,
            tc.tile_pool(name="dram", bufs=1, space="DRAM") as dram,
        ):
            # ---- constants to SBUF
            eidx_t = cpool.tile([P, NSL], i32, name="eidx_t")
            dstloc_t = cpool.tile([P, NSL], f32, name="dstloc_t")
            dinv_t = cpool.tile([P, NW], f32, name="dinv_t")
            w0_t = cpool.tile([P, H], f32, name="w0_t")
            wp09_t = cpool.tile([P, L * P], f32, name="wp09_t")
            wp01_t = cpool.tile([P, L * P], f32, name="wp01_t")
            bn_s_t = cpool.tile([P, L * P], f32, name="bn_s_t")
            bn_b_t = cpool.tile([P, L * P], f32, name="bn_b_t")
            b0r_t = cpool.tile([P, H], f32, name="b0r_t")
            iota_t = cpool.tile([P, P], f32, name="iota_t")
            ident_t = cpool.tile([P, P], f32, name="ident_t")
            for t, io in [(eidx_t, eidx_io), (dstloc_t, dstloc_io),
                          (dinv_t, dinv_io), (w0_t, w0_io), (wp09_t, wp09_io),
                          (wp01_t, wp01_io), (bn_s_t, bn_s_io), (bn_b_t, bn_b_io),
                          (b0r_t, b0r_io), (iota_t, iota_io)]:
                nc.sync.dma_start(t[:], io[:])
            make_identity(nc, ident_t[:])

            x0T_a = bigpool.tile([P, NS_PAD], f32, name="x0T_a")
            x0T_b = bigpool.tile([P, NS_PAD], f32, name="x0T_b")

            tables = [dram.tile([NT, H], f32, addr_space="Shared", name=f"table{i}")
                      for i in range(L + 1)]
            agbufs = [dram.tile([NS_PAD, H], f32, name=f"agbuf{i}")
                      for i in range(L + 1)]
            zsbufs = {i: dram.tile([NS_PAD, H], f32, name=f"zsbuf{i}")
                      for i in (0, 1, 2, 4, 5, 6)}
            hd2buf = dram.tile([NS_PAD, H], f32, name="hd2buf")

            RG = [list(range(C))]

            def allgather(i):
                nc.gpsimd.collective_compute(
                    "AllGather", Alu.bypass, replica_groups=RG,
                    ins=[agbufs[i].opt()], outs=[tables[i].opt()])

            def gather_window(table, w):
                g = gpool.tile([P, n_slots * P], f32, name="g")
                nc.gpsimd.indirect_dma_start(
                    out=g[:], out_offset=None, in_=table[:],
                    in_offset=bass.IndirectOffsetOnAxis(
                        ap=eidx_t[:, w * n_slots:(w + 1) * n_slots], axis=0))
                return g

            def onehot_window(w):
                oh = gpool.tile([P, n_slots * P], f32, name="oh")
                src = dstloc_t[:, w * n_slots:(w + 1) * n_slots]
                in0 = src.to_broadcast([P, n_slots, P])
                io_ap = iota_t[:]
                in1 = bass.AP(io_ap.tensor, io_ap.offset,
                              [list(io_ap.ap[0]), [0, n_slots], [1, P]])
                nc.vector.tensor_tensor(out=oh[:], in0=in0, in1=in1, op=Alu.is_equal)
                return oh

            # ================= Phase A: h' = (x @ w0) * dinv =================
            for w in range(NW):
                ws = slice(w * P, (w + 1) * P)
                xw = wpool.tile([P, H], f32, name="xw")
                nc.sync.dma_start(xw[:], xs_io[ws])
                xT_ps = ps.tile([P, P], f32, name="xT_ps", tag="tr")
                nc.tensor.transpose(out=xT_ps[:], in_=xw[:], identity=ident_t[:])
                xT = wpool.tile([P, P], f32, name="xT")
                nc.vector.tensor_copy(out=xT[:], in_=xT_ps[:])
                h_ps = ps.tile([P, H], f32, name="h_ps", tag="mm")
                nc.tensor.matmul(out=h_ps[:], lhsT=xT[:], rhs=w0_t[:],
                                 start=True, stop=True)
                dcol = dinv_t[:, w:w + 1]
                hp = wpool.tile([P, H], f32, name="hp")
                nc.vector.tensor_scalar_mul(hp[:], h_ps[:], dcol)
                hd2b = wpool.tile([P, H], f32, name="hd2b")
                nc.vector.scalar_tensor_tensor(
                    out=hd2b[:], in0=hp[:], scalar=dcol, in1=b0r_t[:],
                    op0=Alu.mult, op1=Alu.add)
                nc.sync.dma_start(agbufs[0][ws], hp[:])
                nc.scalar.dma_start(hd2buf[ws], hd2b[:])
            allgather(0)

            # ============ Phase B: z0 = dinv*segsum(h'[src]) + h*dinv^2 + b0
            for w in range(NW):
                ws = slice(w * P, (w + 1) * P)
                g = gather_window(tables[0], w)
                oh = onehot_window(w)
                s_ps = ps.tile([P, H], f32, name="s_ps", tag="acc")
                for ci in range(n_slots):
                    cs = slice(ci * P, (ci + 1) * P)
                    nc.tensor.matmul(out=s_ps[:], lhsT=oh[:, cs], rhs=g[:, cs],
                                     start=(ci == 0), stop=(ci == n_slots - 1))
                hd2w = wpool.tile([P, H], f32, name="hd2w")
                nc.scalar.dma_start(hd2w[:], hd2buf[ws])
                z0 = wpool.tile([P, H], f32, name="z0")
                nc.vector.scalar_tensor_tensor(
                    out=z0[:], in0=s_ps[:], scalar=dinv_t[:, w:w + 1], in1=hd2w[:],
                    op0=Alu.mult, op1=Alu.add)
                nc.sync.dma_start(agbufs[1][ws], z0[:])
                zT_ps = ps.tile([P, P], f32, name="zT_ps", tag="tr")
                nc.tensor.transpose(out=zT_ps[:], in_=z0[:], identity=ident_t[:])
                nc.vector.tensor_copy(out=x0T_a[:, ws], in_=zT_ps[:])
            allgather(1)

            # =========================== Phase C: 8 GCN2 layers
            for i in range(L):
                x0T = x0T_a if i < 4 else x0T_b
                lsl = slice(i * P, (i + 1) * P)
                for w in range(NW):
                    ws = slice(w * P, (w + 1) * P)
                    g = gather_window(tables[i + 1], w)
                    oh = onehot_window(w)
                    st_ps = ps.tile([P, P], f32, name="st_ps", tag="acc")
                    for ci in range(n_slots):
                        cs = slice(ci * P, (ci + 1) * P)
                        nc.tensor.matmul(out=st_ps[:], lhsT=g[:, cs], rhs=oh[:, cs],
                                         start=(ci == 0), stop=(ci == n_slots - 1))
                    st = wpool.tile([P, P], f32, name="st")
                    nc.vector.tensor_copy(out=st[:], in_=st_ps[:])
                    z_ps = ps.tile([P, H], f32, name="z_ps", tag="mm")
                    nc.tensor.matmul(out=z_ps[:], lhsT=st[:], rhs=wp09_t[:, lsl],
                                     start=True, stop=False)
                    nc.tensor.matmul(out=z_ps[:], lhsT=x0T[:, ws], rhs=wp01_t[:, lsl],
                                     start=False, stop=True)
                    if i in (3, 7):
                        m = wpool.tile([P, H], f32, name="m")
                        nc.vector.tensor_copy(out=m[:], in_=z_ps[:])
                        for j in range(4 * (i // 4), 4 * (i // 4) + 3):
                            zl = wpool.tile([P, H], f32, name="zl")
                            nc.scalar.dma_start(zl[:], zsbufs[j][ws])
                            nc.vector.tensor_max(m[:], m[:], zl[:])
                        if i == 3:
                            nc.sync.dma_start(agbufs[i + 2][ws], m[:])
                            mT_ps = ps.tile([P, P], f32, name="mT_ps", tag="tr")
                            nc.tensor.transpose(out=mT_ps[:], in_=m[:],
                                                identity=ident_t[:])
                            nc.vector.tensor_copy(out=x0T_b[:, ws], in_=mT_ps[:])
                        else:
                            nc.sync.dma_start(out_io[ws], m[:])
                    else:
                        zsb = wpool.tile([P, H], f32, name="zsb")
                        nc.vector.tensor_copy(out=zsb[:], in_=z_ps[:])
                        nc.scalar.dma_start(zsbufs[i][ws], zsb[:])
                        t1 = wpool.tile([P, H], f32, name="t1")
                        nc.vector.tensor_tensor(out=t1[:], in0=z_ps[:],
                                                in1=bn_s_t[:, lsl], op=Alu.mult)
                        t2 = wpool.tile([P, H], f32, name="t2")
                        nc.vector.tensor_tensor(out=t2[:], in0=t1[:],
                                                in1=bn_b_t[:, lsl], op=Alu.add)
                        za = wpool.tile([P, H], f32, name="za")
                        nc.scalar.activation(out=za[:], in_=t2[:], func=Act.Relu)
                        nc.sync.dma_start(agbufs[i + 2][ws], za[:])
                if i < 7:
                    allgather(i + 2)
    nc.finalize()
    return nc


_PROGRAM_CACHE = {}
_PREP_CACHE = {}


def kernel(**inputs) -> np.ndarray:
    from concourse.bass_utils import run_bass_kernel_spmd

    edge_index = np.asarray(inputs["edge_index"])
    ekey = hashlib.md5(edge_index.tobytes()).hexdigest()
    if ekey not in _PREP_CACHE:
        _PREP_CACHE[ekey] = _host_prep(edge_index)
    prep = _PREP_CACHE[ekey]
    n_slots = prep["n_slots"]
    if n_slots not in _PROGRAM_CACHE:
        _PROGRAM_CACHE[n_slots] = _build_program(n_slots)
    nc = _PROGRAM_CACHE[n_slots]

    consts = _host_consts(inputs)
    x = np.asarray(inputs["x"], np.float32)
    xp = np.zeros((C * NS_PAD, H), np.float32)
    xp[prep["perm_pos"]] = x

    in_maps = []
    for c in range(C):
        in_maps.append({
            "xs": xp[c * NS_PAD:(c + 1) * NS_PAD],
            "eidx": prep["eidx"][c],
            "dstloc": prep["dstloc"][c],
            "dinv": prep["dinv"][c],
            "w0": consts["w0"], "wp09": consts["wp09"], "wp01": consts["wp01"],
            "bn_s": consts["bn_s"], "bn_b": consts["bn_b"], "b0r": consts["b0r"],
            "iota": consts["iota"],
        })
    res = run_bass_kernel_spmd(nc, in_maps, list(range(C)))
    out_cat = np.concatenate([res.results[c]["out"] for c in range(C)], axis=0)
    return out_cat[prep["perm_pos"]]
